# revision 100
# baseline (speedup 1.0000x reference)
"""Trainium2 8-core Bass kernel for the UniGAT hypergraph attention block.

Algorithm (matches the jax reference numerically, up to bf16 rounding):
  1. Xh = X @ theta_cat + b          (per-core node shard, PE matmul)
  2. v2e: esum[e] = sum over incidence pairs (e,v) of Xh[v]
       - per-core partial over its node shard: dma_gather of Xh rows per
         pair (sorted by edge) + 0/1-indicator segment matmul on PE
       - AllReduce(esum) over the 8 cores
  3. Softmax folding: w = exp(s)/sum(exp(s)) exactly (the segment-max
     subtraction cancels; s = leaky_relu in [-0.5, 0.5] so exp is safe).
     Build per-edge table Z = [Y*expS | expS] where Y = esum*inv_cnt,
     expS[e,h] = exp(leaky_relu(inv_cnt*(esum @ aw_h))).
  4. e2v: plain 0/1 segment-sum of gathered Z rows per destination vertex
     (sorted by vertex) -> numerator (256 cols) and denominator (4 cols);
     divide per head.
  5. ELU -> LayerNorm -> GELU -> conv matmul -> X + gamma * Xo.

Sharding: nodes (and pairs grouped by destination vertex) across 8 cores;
weights and edge tables replicated; one AllReduce of esum is the only
collective.
"""

import os

import numpy as np
import ml_dtypes

import concourse.bass as bass
import concourse.bacc as bacc
import concourse.tile as tile
import concourse.mybir as mybir
from concourse.bass_utils import run_bass_kernel_spmd
from concourse.library_config import mlp

BF16 = mybir.dt.bfloat16
F8 = mybir.dt.float8e4
F32 = mybir.dt.float32
I16 = mybir.dt.int16
AL = mybir.AluOpType
AF = mybir.ActivationFunctionType

P = 128
NCORES = 8

N_NODES = 100000
N_EDGES = 20000
NNZ = 500000
CH = 256
H = 4
DH = 64
NEG_SLOPE = 0.2
LN_EPS = 1e-6

NPC = N_NODES // NCORES          # 12500
VG = (NPC + P - 1) // P          # 98
NPC_PAD = VG * P                 # 12544
EG = 160                         # edge groups, padded so every ReduceScatter
                                 # chunk splits into 8 whole groups per core
E_PAD = EG * P                   # 20480

ZW = 512                         # Z table row stride in fp8 bytes (260 used)
GATHER_CALL_V2E = 4096
GATHER_CALL_E2V = 2048
NQ = 4                           # SWDGE queues (HW max 4); each queue has its
                                 # own Q7 core pair + DMA ring, so gathers on
                                 # different queues run concurrently.
VPREP = 6                        # v2e gather tiles ring depth
VTRIG = 2                        # v2e gather emission lookahead
EPREP = 6                        # e2v gather tiles ring depth
ETRIG = 2
SELW = 16                        # indicator chunks built per DVE op
CHUNK_POS = [32, 64, 96, 128, 152, EG]   # esum AllReduce chunk ends (group
                                         # index, multiples of 4); early small
                                         # chunks start the CC sooner, tiny
                                         # tail chunk keeps the last AR short


def _bf(x):
    return np.asarray(x, dtype=ml_dtypes.bfloat16)


def _f8(x):
    return np.asarray(x, dtype=ml_dtypes.float8_e4m3)


def _wrap16(idx):
    """dma_gather index layout: index i -> [i % 16, i // 16], replicated x8."""
    assert idx.size % 16 == 0
    w = idx.reshape(-1, 16).T
    return np.ascontiguousarray(np.tile(w, (8, 1))).astype(np.int16)


def _pairmajor(vals, dtype):
    """pair i -> [i % 128, i // 128]."""
    assert vals.size % P == 0
    return np.ascontiguousarray(vals.reshape(-1, P).T).astype(dtype)


def _sel_bytes(rel):
    """rel [M, P] (0..127 valid, 255 pad) -> fp8 one-hot sel tensor
    [P(slot), M*P] with [p, m*128+d] = (rel[m, p] == d)."""
    M = rel.shape[0]
    sel = np.zeros((M, P, P), dtype=ml_dtypes.float8_e4m3)
    m_i, p_i = np.nonzero(rel != 255.0)
    sel[m_i, p_i, rel[m_i, p_i].astype(np.int64)] = 1.0
    return np.ascontiguousarray(sel.transpose(1, 0, 2)).reshape(P, M * P)


def _balance_groups(deg):
    """Assign NPC vertices to VG groups of <=128, balancing pair sums.
    Returns perm: old local id -> new local id."""
    import heapq
    order = np.argsort(-deg, kind="stable")
    heap = [(0, 0, g) for g in range(VG)]
    heapq.heapify(heap)
    perm = np.zeros(NPC, dtype=np.int64)
    fill = np.zeros(VG, dtype=np.int64)
    for v in order:
        while True:
            s, cnt, g = heapq.heappop(heap)
            if cnt < P:
                perm[v] = g * P + fill[g]
                fill[g] += 1
                heapq.heappush(heap, (s + int(deg[v]), cnt + 1, g))
                break
    return perm


def _build_streams_unaligned(vals, rels, cnts, order=None):
    """Common unaligned layout in the given group processing order: the i-th
    processed group order[i] occupies slots [S_i, S_i+cap_i). Returns idx
    [NCORES,T], rel columns per matmul entry [NCORES, M, P], and per-POSITION
    entry chunk lists."""
    ngroups = cnts.shape[1]
    if order is None:
        order = list(range(ngroups))
    caps = np.maximum(cnts.max(axis=0), 1)
    starts = np.concatenate([[0], np.cumsum(caps[order])])
    T = int(-(-starts[-1] // P) * P)
    entries = []            # (pos, k) in emission order
    group_ks = []
    for i in range(ngroups):
        k0 = int(starts[i] // P)
        k1 = int((starts[i + 1] - 1) // P)
        ks = list(range(k0, k1 + 1))
        group_ks.append(ks)
        entries.extend((i, k) for k in ks)
    M = len(entries)
    idx_s = np.zeros((NCORES, T), dtype=np.int64)
    rel_s = np.full((NCORES, M, P), 255.0, dtype=np.float32)
    for c in range(NCORES):
        gstart = np.concatenate([[0], np.cumsum(cnts[c])])
        for i in range(ngroups):
            g = order[i]
            n = int(cnts[c][g])
            if n == 0:
                continue
            s, d = int(gstart[g]), int(starts[i])
            idx_s[c, d:d + n] = vals[c][s:s + n]
        for m, (i, k) in enumerate(entries):
            g = order[i]
            lo = max(int(starts[i]), k * P)
            hi = min(int(starts[i]) + int(cnts[c][g]), (k + 1) * P)
            if hi <= lo:
                continue
            s = int(gstart[g]) + (lo - int(starts[i]))
            rel_s[c, m, lo - k * P:hi - k * P] = rels[c][s:s + hi - lo] % P
    return idx_s, rel_s, group_ks, T, M


def _balance_edge_groups(edge_idx, core):
    """Renumber edges so each 128-edge group has balanced per-core pair
    counts: the v2e gather stream is padded to sum_g max_c(count), so
    balancing trims gathered rows. Greedy LPT on the [group, core] loads."""
    pc = np.bincount(core * N_EDGES + edge_idx,
                     minlength=NCORES * N_EDGES).reshape(NCORES, N_EDGES)
    tot = pc.sum(0)
    order = np.argsort(-tot, kind="stable")
    loads = np.zeros((EG, NCORES))
    fill = np.zeros(EG, dtype=np.int64)
    eperm = np.empty(N_EDGES, dtype=np.int64)
    for e in order:
        cand = (loads + pc[:, e][None, :]).max(axis=1)
        cand[fill >= P] = np.inf
        g = int(np.argmin(cand))
        eperm[e] = g * P + fill[g]
        fill[g] += 1
        loads[g] += pc[:, e]
    return eperm


def make_plan(edge_idx, vertex_idx):
    """Host-side index preprocessing (graph structure only)."""
    edge_idx = np.asarray(edge_idx).astype(np.int64)
    vertex_idx = np.asarray(vertex_idx).astype(np.int64)
    core = vertex_idx // NPC
    lv = vertex_idx - core * NPC
    # edge renumbering: balances per-core loads within each edge group
    eperm = _balance_edge_groups(edge_idx, core)
    edge_idx = eperm[edge_idx]

    v2e_e, v2e_lv = [], []
    e2v_e, e2v_nlv = [], []
    perms = []
    for c in range(NCORES):
        m = core == c
        e_c, lv_c = edge_idx[m], lv[m]
        o = np.argsort(e_c, kind="stable")
        v2e_e.append(e_c[o])
        v2e_lv.append(lv_c[o])
        deg = np.bincount(lv_c, minlength=NPC)
        perm = _balance_groups(deg)
        perms.append(perm)
        nlv_c = perm[lv_c]
        o = np.argsort(nlv_c, kind="stable")
        e2v_e.append(e_c[o])
        e2v_nlv.append(nlv_c[o])

    def group_counts(keys_list, ngroups):
        cnts = np.zeros((NCORES, ngroups), dtype=np.int64)
        for c in range(NCORES):
            cnts[c] = np.bincount(keys_list[c] // P, minlength=ngroups)
        return cnts

    v2e_cnts = group_counts(v2e_e, EG)
    e2v_cnts = group_counts(e2v_nlv, VG)

    v2e_idx, v2e_rel, v2e_gks, v2e_T, v2e_M = _build_streams_unaligned(
        v2e_lv, v2e_e, v2e_cnts)
    e2v_idx, e2v_rel, e2v_gks, e2v_T, e2v_M = _build_streams_unaligned(
        e2v_e, e2v_nlv, e2v_cnts)

    cnt = np.bincount(edge_idx, minlength=E_PAD).astype(np.float32)
    inv_cnt = 1.0 / np.maximum(cnt, 1.0)

    return dict(
        v2e_gks=v2e_gks, e2v_gks=e2v_gks,
        v2e_T=v2e_T, e2v_T=e2v_T, v2e_M=v2e_M, e2v_M=e2v_M,
        v2e_idx=v2e_idx, v2e_rel=v2e_rel,
        e2v_idx=e2v_idx, e2v_rel=e2v_rel,
        perms=perms,
        inv_cnt=inv_cnt,
    )


def _n_gather_calls(total_chunks, call_pairs):
    total = total_chunks * P
    n_full, rem = divmod(total, call_pairs)
    sizes = [call_pairs] * n_full
    if rem:
        sizes.append(rem)
    return sizes


def build_kernel(v2e_gks, e2v_gks, v2e_T, e2v_T, v2e_M, e2v_M,
                 debug_tables=False, ln_trivial=False):
    v2e_tot = v2e_T
    e2v_tot = e2v_T

    nc = bacc.Bacc("TRN2", target_bir_lowering=False, debug=False,
                   num_devices=NCORES, num_swdge_queues=NQ,
                   dynamic_dma_scratch_size=32768)

    x_in = nc.dram_tensor("x", [NPC_PAD, CH], BF16, kind="ExternalInput")
    xt_in = nc.dram_tensor("xt", [P, VG * 2 * P], BF16, kind="ExternalInput")
    wcat_in = nc.dram_tensor("wcat", [CH, CH], BF16, kind="ExternalInput")
    convw_in = nc.dram_tensor("convw", [CH, CH], BF16, kind="ExternalInput")
    browbf_in = nc.dram_tensor("browbf", [1, CH], BF16, kind="ExternalInput")
    awrep_in = nc.dram_tensor("awrep", [P, CH], BF16, kind="ExternalInput")
    convbrep_in = nc.dram_tensor("convbrep", [P, CH], F32, kind="ExternalInput")
    gammarep_in = nc.dram_tensor("gammarep", [P, CH], F32, kind="ExternalInput")
    lnwrep_in = nc.dram_tensor("lnwrep", [P, CH], F32, kind="ExternalInput")
    lnbrep_in = nc.dram_tensor("lnbrep", [P, CH], F32, kind="ExternalInput")
    iota_in = nc.dram_tensor("iota", [P, P], BF16, kind="ExternalInput")
    ident_in = nc.dram_tensor("ident", [P, P], BF16, kind="ExternalInput")
    invc_in = nc.dram_tensor("invc", [P, EG], F32, kind="ExternalInput")
    c14_in = nc.dram_tensor("c14", [P, H], F32, kind="ExternalInput")
    epscol_in = nc.dram_tensor("epscol", [P, 1], F32, kind="ExternalInput")
    v2ei_in = nc.dram_tensor("v2ei", [P, v2e_tot // 16], I16, kind="ExternalInput")
    v2er_in = nc.dram_tensor("v2er", [P, v2e_M], BF16, kind="ExternalInput")
    e2vi_in = nc.dram_tensor("e2vi", [P, e2v_tot // 16], I16, kind="ExternalInput")
    e2vr_in = nc.dram_tensor("e2vr", [P, e2v_M], BF16, kind="ExternalInput")
    # f32: the gamma-scaled (1e-6) GNN term is below bf16 resolution vs X,
    # so a bf16 output would silently drop it.
    out_ext = nc.dram_tensor("out", [NPC_PAD, CH], F32, kind="ExternalOutput")
    dbg = {}
    if debug_tables:
        dbg["xh"] = nc.dram_tensor("dbg_xh", [NPC_PAD, CH], F8, kind="ExternalOutput")
        dbg["z"] = nc.dram_tensor("dbg_z", [E_PAD, ZW], F8, kind="ExternalOutput")
        dbg["xn"] = nc.dram_tensor("dbg_xn", [NPC_PAD, CH], BF16, kind="ExternalOutput")

    def rows(dr, t0, w):
        return dr[t0 * P:(t0 + w) * P, :].rearrange("(t p) f -> p t f", p=P)

    with tile.TileContext(nc) as tc:
        with tc.tile_pool(name="dram", bufs=1, space="DRAM") as dram, \
             tc.tile_pool(name="const", bufs=1) as cpool, \
             tc.tile_pool(name="resident", bufs=1) as rpool:

            nc.gpsimd.load_library(mlp)

            xh_table = dram.tile([NPC_PAD, CH], F8)
            esum_bounce = dram.tile([E_PAD, CH], BF16)
            # esum chunks: ReduceScatter gives each core 1/8 of the chunk's
            # groups; the core computes z for its shard and AllGathers the z
            # rows back into the (Shared) z_table. Chunk bounds are multiples
            # of 4 (esum is evacuated in 4-group batches) and chunk sizes are
            # multiples of 8 (whole groups per core).
            _ch_lims = list(zip([0] + CHUNK_POS[:-1], CHUNK_POS))
            yfulls = []
            for _ci, (_a, _b) in enumerate(_ch_lims):
                yf = dram.tile([(_b - _a) * P, CH], BF16, addr_space="Shared",
                               name=f"yfull{_ci}", tag=f"yfull{_ci}")
                yfulls.append(yf)
            z_table = dram.tile([E_PAD, ZW], F8)

            def cload(dr, shape, dtype, name):
                t = cpool.tile(shape, dtype, name=name, tag=name)
                nc.sync.dma_start(t[:], dr[:])
                return t

            w_sb = cpool.tile([P, 2, CH], BF16)
            nc.sync.dma_start(w_sb[:], wcat_in[:].rearrange("(k p) f -> p k f", p=P))
            convw_sb = cpool.tile([P, 2, CH], BF16)
            nc.sync.dma_start(convw_sb[:], convw_in[:].rearrange("(k p) f -> p k f", p=P))
            browbf = cload(browbf_in, [1, CH], BF16, "browbf")
            ones1 = cpool.tile([1, P], BF16)
            nc.vector.memset(ones1[:], 1.0)
            awrep = cload(awrep_in, [P, CH], BF16, "awrep")
            convbrep = cload(convbrep_in, [P, CH], F32, "convbrep")
            gammarep = cload(gammarep_in, [P, CH], F32, "gammarep")
            lnwrep = cload(lnwrep_in, [P, CH], F32, "lnwrep")
            lnbrep = cload(lnbrep_in, [P, CH], F32, "lnbrep")
            iota = cload(iota_in, [P, P], BF16, "iota")
            ident = cload(ident_in, [P, P], BF16, "ident")
            invc = cload(invc_in, [P, EG], F32, "invc")
            c14 = cload(c14_in, [P, H], F32, "c14")
            epscol = cload(epscol_in, [P, 1], F32, "epscol")
            gcb = cpool.tile([P, CH], F32)
            nc.vector.tensor_tensor(out=gcb[:], in0=gammarep[:], in1=convbrep[:],
                                    op=AL.mult)
            # Warmup collective: the first AllReduce pays ~150us of CC mesh
            # init; absorb it here, overlapped with phase 1.
            warm_in = dram.tile([P, CH], BF16)
            warm_out = dram.tile([P, CH], BF16, addr_space="Shared",
                                 name="warmout", tag="warmout")
            wt = cpool.tile([P, CH], BF16)
            nc.vector.memset(wt[:], 0.0)
            nc.sync.dma_start(warm_in[:], wt[:])
            nc.gpsimd.collective_compute(
                "AllReduce", AL.add, replica_groups=[list(range(NCORES))],
                ins=[warm_in[:].opt()], outs=[warm_out[:].opt()])
            # fold gamma into conv_w columns: (Xg @ W) * gamma = Xg @ (W * gamma_row)
            nc.vector.tensor_tensor(
                out=convw_sb[:], in0=convw_sb[:],
                in1=gammarep[:, None, :].to_broadcast([P, 2, CH]), op=AL.mult)

            def z_chunk(zpool, ci, t_lo, t_hi):
                """z for groups [t_lo, t_hi) (chunk-local) of chunk ci: read
                yfulls[ci], write z_table rows directly (replicated work)."""
                a, _ = _ch_lims[ci]
                for t0 in range(t_lo, t_hi, 8):
                    w = min(8, t_hi - t0)
                    c0 = a + t0
                    y4 = zpool.tile([P, 8, CH], BF16, tag="zy")
                    nc.sync.dma_start(
                        y4[:, :w, :],
                        yfulls[ci][t0 * P:(t0 + w) * P, :].rearrange(
                            "(t p) f -> p t f", p=P))
                    tmp = zpool.tile([P, 8, CH], BF16, tag="ztmp")
                    nc.vector.tensor_tensor(
                        out=tmp[:, :w, :], in0=y4[:, :w, :],
                        in1=awrep[:, None, :].to_broadcast([P, w, CH]), op=AL.mult)
                    beta = zpool.tile([P, 8, H], F32, tag="zbeta")
                    nc.vector.tensor_reduce(
                        out=beta[:, :w, :],
                        in_=tmp[:, :w, :].rearrange("p t (h d) -> p t h d", d=DH),
                        axis=mybir.AxisListType.X, op=AL.add)
                    al_ = zpool.tile([P, 8, H], F32, tag="zal")
                    nc.vector.tensor_tensor(
                        out=al_[:, :w, :], in0=beta[:, :w, :],
                        in1=invc[:, c0:c0 + w, None].to_broadcast([P, w, H]),
                        op=AL.mult)
                    sal = zpool.tile([P, 8, H], F32, tag="zsal")
                    nc.scalar.activation(out=sal[:, :w, :], in_=al_[:, :w, :],
                                         func=AF.Prelu, alpha=NEG_SLOPE)
                    zrow = zpool.tile([P, 8, CH + H], F8, tag="zrow")
                    expS = zrow[:, :w, CH:CH + H]
                    nc.scalar.activation(out=expS, in_=sal[:, :w, :], func=AF.Exp)
                    s4 = zpool.tile([P, 8, H], F32, tag="zs4")
                    nc.vector.tensor_tensor(
                        out=s4[:, :w, :], in0=expS,
                        in1=invc[:, c0:c0 + w, None].to_broadcast([P, w, H]),
                        op=AL.mult)
                    nc.vector.tensor_tensor(
                        out=zrow[:, :w, :CH].rearrange("p t (h d) -> p t h d", d=DH),
                        in0=y4[:, :w, :].rearrange("p t (h d) -> p t h d", d=DH),
                        in1=s4[:, :w, :, None].to_broadcast([P, w, H, DH]),
                        op=AL.mult)
                    nc.sync.dma_start(
                        z_table[c0 * P:(c0 + w) * P, :CH + H].rearrange(
                            "(t p) f -> p t f", p=P),
                        zrow[:, :w, :])


            # ================= Phases 1+2 =================
            # v2e gather pools open around phase 1 so the SWDGE descriptor
            # preps (pure Q7 work, no xh dependency) run during the Xh matmul;
            # the triggers carry the deferred RAW on xh_table.
            with tc.tile_pool(name="v2esb", bufs=VPREP) as gpool, \
                 tc.tile_pool(name="v2esel", bufs=8) as selpool, \
                 tc.tile_pool(name="v2eev", bufs=3) as evpool, \
                 tc.tile_pool(name="v2eidx", bufs=1) as ipool, \
                 tc.tile_pool(name="zsbv", bufs=3) as zpool_v2e, \
                 tc.tile_pool(name="v2eps", bufs=4, space="PSUM") as v2eps:
                v2ei = ipool.tile([P, v2e_tot // 16], I16)
                nc.sync.dma_start(v2ei[:], v2ei_in[:])
                v2er = ipool.tile([P, v2e_M], BF16)
                nc.sync.dma_start(v2er[:], v2er_in[:])
                call_sizes = _n_gather_calls(v2e_tot // P, GATHER_CALL_V2E)
                ncalls = len(call_sizes)
                gtiles = [None] * ncalls
                v2e_state = dict(trig=0)

                def v2e_trigger_to(gc):
                    while v2e_state["trig"] <= gc:
                        t = v2e_state["trig"]
                        n = call_sizes[t]
                        gt = gpool.tile([P, GATHER_CALL_V2E // P, CH], F8,
                                        tag="v2egather")
                        s = t * GATHER_CALL_V2E
                        nc.gpsimd.dma_gather(
                            gt[:, :n // P, :], xh_table[:],
                            v2ei[:, s // 16:(s + n) // 16], n, n, CH,
                            single_packet=False, queue_num=t % NQ)
                        gtiles[t] = gt
                        v2e_state["trig"] = t + 1

                # ---- Phase 1: Xh = X @ W + b (PE/scalar; Pool runs preps) --
                with tc.tile_pool(name="p1sb", bufs=3) as p1sb, \
                     tc.tile_pool(name="p1xt", bufs=1) as p1xt, \
                     tc.tile_pool(name="p1ps", bufs=2, space="PSUM") as p1ps:
                    xt_sb = p1xt.tile([P, VG * 2 * P], BF16, tag="xt")
                    XTC = VG * 2 * P // 4
                    assert XTC * 4 == VG * 2 * P
                    for c4 in range(4):
                        nc.sync.dma_start(xt_sb[:, c4 * XTC:(c4 + 1) * XTC],
                                          xt_in[:, c4 * XTC:(c4 + 1) * XTC])
                    xt_v = xt_sb[:].rearrange("p (t k f) -> p t k f", t=VG, k=2)
                    xh8, t0, tw = None, 0, 0
                    psf = None
                    for t in range(VG):
                        # two row-tiles share one PSUM bank; one fat copy per
                        # pair, one DMA per 8 tiles.
                        half = t % 2
                        if half == 0:
                            psf = p1ps.tile([P, 512], F32, tag="xhps")
                        ps = psf[:, half * CH:(half + 1) * CH]
                        for k in range(2):
                            nc.tensor.matmul(ps, lhsT=xt_v[:, t, k, :],
                                             rhs=w_sb[:, k, :],
                                             start=(k == 0), stop=False)
                        nc.tensor.matmul(ps, lhsT=ones1[:], rhs=browbf[:],
                                         start=False, stop=True)
                        if t % 8 == 0:
                            t0 = t
                            tw = min(8, VG - t0)
                            xh8 = p1sb.tile([P, 8, CH], F8, tag="xhout")
                        if half == 1 or t == VG - 1:
                            w2 = half + 1
                            pr0 = t - t0 - half
                            if (t - t0) % 4 < 2:
                                nc.scalar.copy(
                                    out=xh8[:, pr0:pr0 + w2, :].rearrange(
                                        "p t f -> p (t f)"),
                                    in_=psf[:, :w2 * CH])
                            else:
                                nc.vector.tensor_copy(
                                    xh8[:, pr0:pr0 + w2, :].rearrange(
                                        "p t f -> p (t f)"),
                                    psf[:, :w2 * CH])
                        if t - t0 == tw - 1:
                            nc.sync.dma_start(rows(xh_table, t0, tw), xh8[:, :tw, :])
                    if debug_tables:
                        nc.sync.dma_start(dbg["xh"][:], xh_table[:])

                # ---- Phase 2: v2e partial esum ----
                sel_cur, sel0 = None, 0
                esb4, e0, ew = None, 0, 0
                ent = 0
                # z strips for chunk ci are emitted LAG groups after its
                # AllReduce so the (in-order) DVE queue doesn't stall on the
                # collective; strips of 8 tiles every 3 groups.
                LAG = 40
                _zq = {}
                for _ci, (_a, _b) in enumerate(_ch_lims):
                    for _si, _t0 in enumerate(range(0, _b - _a, 8)):
                        _zq.setdefault(CHUNK_POS[_ci] + LAG + 3 * _si,
                                       []).append((_ci, _t0,
                                                   min(_t0 + 8, _b - _a)))
                cs, ce = 0, CHUNK_POS[0]
                for i in range(EG):
                    if i in CHUNK_POS:
                        cs = i
                        ce = CHUNK_POS[CHUNK_POS.index(i) + 1]
                    for _ci, _a, _b in _zq.get(i, []):
                        z_chunk(zpool_v2e, _ci, _a, _b)
                    psf = v2eps.tile([P, 512], F32, tag="v2eps")
                    ps = psf[:, :CH]
                    ks = v2e_gks[i]
                    for ii, k in enumerate(ks):
                        gc, j = divmod(k, GATHER_CALL_V2E // P)
                        v2e_trigger_to(min(gc + VTRIG, ncalls - 1))
                        if ent % SELW == 0:
                            sel0 = ent
                            sw = min(SELW, v2e_M - ent)
                            sel_cur = selpool.tile([P, SELW, P], F8, tag="v2esel")
                            nc.vector.tensor_tensor(
                                out=sel_cur[:, :sw, :],
                                in0=v2er[:, ent:ent + sw, None].to_broadcast(
                                    [P, sw, P]),
                                in1=iota[:, None, :].to_broadcast([P, sw, P]),
                                op=AL.is_equal)
                        nc.tensor.matmul(ps, lhsT=sel_cur[:, ent - sel0, :],
                                         rhs=gtiles[gc][:, j, :],
                                         start=(ii == 0), stop=(ii == len(ks) - 1))
                        ent += 1
                    if (i - cs) % 4 == 0:
                        e0p = i
                        ew = min(4, ce - e0p)
                        esb4 = evpool.tile([P, 4, CH], BF16, tag="v2eev")
                    nc.scalar.copy(out=esb4[:, i - e0p, :], in_=ps)
                    if i - e0p == ew - 1:
                        nc.sync.dma_start(
                            rows(esum_bounce, e0p, ew),
                            esb4[:, :ew, :])
                    if i + 1 in CHUNK_POS:
                        ci = CHUNK_POS.index(i + 1)
                        a, b = _ch_lims[ci]
                        nc.gpsimd.collective_compute(
                            "AllReduce", AL.add,
                            replica_groups=[list(range(NCORES))],
                            ins=[esum_bounce[a * P:b * P, :].opt()],
                            outs=[yfulls[ci].opt()])
                for i in range(EG, EG + 120):
                    for _ci, _a, _b in _zq.get(i, []):
                        z_chunk(zpool_v2e, _ci, _a, _b)

            if debug_tables:
                with tc.tile_pool(name="dbgz", bufs=1) as _dzp:
                    nc.sync.dma_start(dbg["z"][:], z_table[:])

            # ================= Phase 5: e2v + ELU + LN =================
            with tc.tile_pool(name="e2vsb", bufs=EPREP) as gpool2, \
                 tc.tile_pool(name="e2vsel", bufs=8) as selpool2, \
                 tc.tile_pool(name="e2vev", bufs=2) as evpool2, \
                 tc.tile_pool(name="e2vidx", bufs=1) as ipool2, \
                 tc.tile_pool(name="fsb", bufs=3) as fpool, \
                 tc.tile_pool(name="fps", bufs=2, space="PSUM") as fps, \
                 tc.tile_pool(name="ftps", bufs=2, space="PSUM") as ftps, \
                 tc.tile_pool(name="e2vps", bufs=3, space="PSUM") as e2vps:
                e2vi = ipool2.tile([P, e2v_tot // 16], I16)
                nc.sync.dma_start(e2vi[:], e2vi_in[:])
                e2vr = ipool2.tile([P, e2v_M], BF16)
                nc.sync.dma_start(e2vr[:], e2vr_in[:])
                call_sizes = _n_gather_calls(e2v_tot // P, GATHER_CALL_E2V)
                ncalls2 = len(call_sizes)
                gtiles = [None] * ncalls2
                e2v_state = dict(trig=0)

                def e2v_trigger_to(gc):
                    while e2v_state["trig"] <= gc:
                        t = e2v_state["trig"]
                        n = call_sizes[t]
                        gt = gpool2.tile([P, GATHER_CALL_E2V // P, ZW], F8,
                                         tag="e2vgather")
                        s = t * GATHER_CALL_E2V
                        nc.gpsimd.dma_gather(
                            gt[:, :n // P, :], z_table[:],
                            e2vi[:, s // 16:(s + n) // 16], n, n, ZW,
                            single_packet=False, queue_num=t % NQ)
                        gtiles[t] = gt
                        e2v_state["trig"] = t + 1
                sel_cur, sel0 = None, 0
                elu4, l0, lw = None, 0, 0
                ent = 0
                xc_buf = ipool2.tile([P, VG, CH], BF16)
                var_buf = ipool2.tile([P, VG], F32)
                gcbrow = ipool2.tile([1, CH], BF16)
                nc.vector.tensor_copy(gcbrow[:], gcb[0:1, :])
                _PB_BOUNDS = [16, 32, 48, 64, 80, 84, 88, 92, 96]

                def pass_b(lo, hi):
                    W = hi - lo
                    vb = evpool2.tile([P, 16], F32, tag="vbeps")
                    nc.vector.tensor_scalar_add(vb[:, :W],
                                                var_buf[:, lo:hi], LN_EPS)
                    vrec = evpool2.tile([P, 16], F32, tag="vrec")
                    nc.vector.reciprocal(vrec[:, :W], vb[:, :W])
                    rstd = evpool2.tile([P, 16], F32, tag="rstd")
                    nc.scalar.activation(out=rstd[:, :W],
                                         in_=vrec[:, :W], func=AF.Sqrt)
                    for b0 in range(lo, hi, 4):
                        bw = min(4, hi - b0)
                        xnb = evpool2.tile([P, 4, CH], BF16, tag="xnb")
                        nc.vector.tensor_tensor(
                            out=xnb[:, :bw, :], in0=xc_buf[:, b0:b0 + bw, :],
                            in1=rstd[:, b0 - lo:b0 - lo + bw, None].to_broadcast(
                                [P, bw, CH]),
                            op=AL.mult)
                        if not ln_trivial:
                            nc.vector.tensor_tensor(
                                out=xnb[:, :bw, :], in0=xnb[:, :bw, :],
                                in1=lnwrep[:, None, :].to_broadcast([P, bw, CH]),
                                op=AL.mult)
                            nc.vector.tensor_tensor(
                                out=xnb[:, :bw, :], in0=xnb[:, :bw, :],
                                in1=lnbrep[:, None, :].to_broadcast([P, bw, CH]),
                                op=AL.add)
                        if debug_tables:
                            nc.sync.dma_start(rows(dbg["xn"], b0, bw),
                                              xnb[:, :bw, :])
                        xg4 = fpool.tile([P, 4, CH], BF16, tag="xg4")
                        nc.scalar.activation(out=xg4[:, :bw, :], in_=xnb[:, :bw, :],
                                             func=AF.Gelu)
                        x4 = fpool.tile([P, 4, CH], BF16, tag="x4")
                        nc.sync.dma_start(x4[:, :bw, :], rows(x_in, b0, bw))
                        ofin4 = fpool.tile([P, 4, CH], F32, tag="ofin4")
                        for j in range(bw):
                            xgT = fpool.tile([P, 2, P], BF16, tag="xgT")
                            for k in range(2):
                                tp = ftps.tile([P, P], BF16, tag="tps")
                                nc.tensor.transpose(tp[:],
                                                    xg4[:, j, k * P:(k + 1) * P],
                                                    ident[:])
                                nc.scalar.copy(out=xgT[:, k, :], in_=tp[:])
                            psf2 = fps.tile([P, 512], F32, tag="fps")
                            ps2 = psf2[:, :CH]
                            for k in range(2):
                                nc.tensor.matmul(ps2, lhsT=xgT[:, k, :],
                                                 rhs=convw_sb[:, k, :],
                                                 start=(k == 0), stop=False)
                            # residual X and gamma*conv_b folded into the psum:
                            # ident@x4 adds X rows, ones@gcbrow adds the bias.
                            nc.tensor.matmul(ps2, lhsT=ident[:], rhs=x4[:, j, :],
                                             start=False, stop=False)
                            nc.tensor.matmul(ps2, lhsT=ones1[:], rhs=gcbrow[:],
                                             start=False, stop=True)
                            nc.vector.tensor_copy(ofin4[:, j, :], ps2)
                        nc.sync.dma_start(rows(out_ext, b0, bw), ofin4[:, :bw, :])

                for g in range(VG):
                    psf = e2vps.tile([P, 512], F32, tag="e2vps")
                    ps = psf[:, :CH + H]
                    ks = e2v_gks[g]
                    for i, k in enumerate(ks):
                        gc, j = divmod(k, GATHER_CALL_E2V // P)
                        e2v_trigger_to(min(gc + ETRIG, ncalls2 - 1))
                        if ent % SELW == 0:
                            sel0 = ent
                            sw = min(SELW, e2v_M - ent)
                            sel_cur = selpool2.tile([P, SELW, P], F8, tag="e2vsel")
                            nc.vector.tensor_tensor(
                                out=sel_cur[:, :sw, :],
                                in0=e2vr[:, ent:ent + sw, None].to_broadcast(
                                    [P, sw, P]),
                                in1=iota[:, None, :].to_broadcast([P, sw, P]),
                                op=AL.is_equal)
                        nc.tensor.matmul(ps, lhsT=sel_cur[:, ent - sel0, :],
                                         rhs=gtiles[gc][:, j, :CH + H],
                                         start=(i == 0), stop=(i == len(ks) - 1))
                        ent += 1
                    # xpre = num/den ; ELU(x) = min(exp(x) - 1, relu(x)).
                    # mu and sum-of-squares come from fused accumulators.
                    den = evpool2.tile([P, H], F32, tag="den")
                    nc.vector.tensor_scalar_max(den[:], ps[:, CH:CH + H], 1e-12)
                    rec = evpool2.tile([P, H], F32, tag="rec")
                    nc.vector.reciprocal(rec[:], den[:])
                    xpre = evpool2.tile([P, CH], BF16, tag="xpre")
                    nc.vector.tensor_tensor(
                        out=xpre[:].rearrange("p (h d) -> p h d", d=DH),
                        in0=ps[:, :CH].rearrange("p (h d) -> p h d", d=DH),
                        in1=rec[:, :, None].to_broadcast([P, H, DH]),
                        op=AL.mult)
                    relx = evpool2.tile([P, CH], BF16, tag="relx")
                    nc.scalar.activation(out=relx[:], in_=xpre[:], func=AF.Relu)
                    ep = evpool2.tile([P, CH], F32, tag="ep")
                    nc.scalar.activation(out=ep[:], in_=xpre[:], func=AF.Exp)
                    if g % 4 == 0:
                        l0 = g
                        lw = min(4, VG - l0)
                        elu4 = evpool2.tile([P, 4, CH], BF16, tag="elu4")
                    nc.vector.scalar_tensor_tensor(
                        out=elu4[:, g - l0, :], in0=ep[:], scalar=-1.0, in1=relx[:],
                        op0=AL.add, op1=AL.min)
                    if g - l0 == lw - 1:
                        mu4 = evpool2.tile([P, 4], F32, tag="mu4")
                        nc.vector.tensor_reduce(out=mu4[:, :lw], in_=elu4[:, :lw, :],
                                                axis=mybir.AxisListType.X, op=AL.add)
                        nc.vector.tensor_tensor(out=mu4[:, :lw], in0=mu4[:, :lw],
                                                in1=c14[:, :lw], op=AL.mult)
                        xc4 = xc_buf[:, l0:l0 + lw, :]
                        nc.vector.tensor_tensor(
                            out=xc4, in0=elu4[:, :lw, :],
                            in1=mu4[:, :lw, None].to_broadcast([P, lw, CH]),
                            op=AL.subtract)
                        sq4 = evpool2.tile([P, 4, CH], BF16, tag="sq4")
                        nc.vector.tensor_tensor(out=sq4[:, :lw, :], in0=xc4,
                                                in1=xc4, op=AL.mult)
                        ss4 = evpool2.tile([P, 4], F32, tag="ss4")
                        nc.vector.tensor_reduce(out=ss4[:, :lw], in_=sq4[:, :lw, :],
                                                axis=mybir.AxisListType.X, op=AL.add)
                        nc.vector.tensor_tensor(out=var_buf[:, l0:l0 + lw],
                                                in0=ss4[:, :lw],
                                                in1=c14[:, :lw], op=AL.mult)
                    if (g + 1) in _PB_BOUNDS:
                        pi = _PB_BOUNDS.index(g + 1)
                        pass_b(0 if pi == 0 else _PB_BOUNDS[pi - 1], g + 1)
                if _PB_BOUNDS[-1] < VG:
                    pass_b(_PB_BOUNDS[-1], VG)

    nc.compile()
    return nc


def prepare_inputs(X, edge_idx, vertex_idx, theta_w, theta_b, atten_w,
                   ln_w, ln_b, conv_w, conv_b, gamma, plan):
    X = np.asarray(X, dtype=np.float32)
    theta_w = np.asarray(theta_w, dtype=np.float32)
    wcat = _bf(theta_w.transpose(1, 0, 2).reshape(CH, CH))
    browbf = _bf(np.asarray(theta_b, np.float32).reshape(1, CH))
    awrep = _bf(np.tile(np.asarray(atten_w, np.float32).reshape(1, CH), (P, 1)))
    convw = _bf(np.asarray(conv_w, np.float32))
    convbrep = np.tile(np.asarray(conv_b, np.float32).reshape(1, CH), (P, 1))
    gammarep = np.tile(np.asarray(gamma, np.float32).reshape(1, CH), (P, 1))
    lnwrep = np.tile(np.asarray(ln_w, np.float32).reshape(1, CH), (P, 1))
    lnbrep = np.tile(np.asarray(ln_b, np.float32).reshape(1, CH), (P, 1))
    iota = _bf(np.tile(np.arange(P, dtype=np.float32), (P, 1)))
    ident = _bf(np.eye(P, dtype=np.float32))
    invc_full = np.ascontiguousarray(
        plan["inv_cnt"].reshape(EG, P).T).astype(np.float32)
    c14 = np.full((P, H), 1.0 / CH, np.float32)
    epscol = np.full((P, 1), LN_EPS, np.float32)

    in_maps = []
    for c in range(NCORES):
        xc = np.zeros((NPC_PAD, CH), np.float32)
        xc[:NPC] = X[c * NPC:(c + 1) * NPC]
        xcb = _bf(xc)
        # xt / xh_table stay in original local order (v2e gathers by old lv);
        # the residual input x follows the e2v output renumbering.
        xt = np.ascontiguousarray(
            _bf(xc).reshape(VG, P, 2, P).transpose(3, 0, 2, 1)).reshape(
                P, VG * 2 * P)
        perm = plan["perms"][c]
        xp = np.zeros((NPC_PAD, CH), np.float32)
        xp[perm] = xc[:NPC]
        in_maps.append(dict(
            x=_bf(xp), xt=xt, wcat=wcat, convw=convw,
            browbf=browbf, awrep=awrep,
            convbrep=convbrep.astype(np.float32),
            gammarep=gammarep.astype(np.float32),
            lnwrep=lnwrep.astype(np.float32), lnbrep=lnbrep.astype(np.float32),
            iota=iota, ident=ident, invc=invc_full, c14=c14, epscol=epscol,
            v2ei=_wrap16(plan["v2e_idx"][c]),
            v2er=np.ascontiguousarray(
                plan["v2e_rel"][c].T).astype(ml_dtypes.bfloat16),
            e2vi=_wrap16(plan["e2v_idx"][c]),
            e2vr=np.ascontiguousarray(
                plan["e2v_rel"][c].T).astype(ml_dtypes.bfloat16),
        ))
    return in_maps


_CACHE = {}


def kernel(X, edge_idx, vertex_idx, theta_w, theta_b, atten_w,
           ln_w, ln_b, conv_w, conv_b, gamma):
    debug_tables = bool(int(os.environ.get("GNN_DEBUG_TABLES", "0")))
    trace = bool(int(os.environ.get("GNN_TRACE", "0")))

    plan = make_plan(edge_idx, vertex_idx)
    # LayerNorm affine is identity for this model's inputs; skip the two
    # elementwise ops when so (general path kept for other values).
    ln_trivial = bool(np.allclose(np.asarray(ln_w), 1.0)
                      and np.allclose(np.asarray(ln_b), 0.0))
    key = (tuple(map(tuple, plan["v2e_gks"])), tuple(map(tuple, plan["e2v_gks"])),
           debug_tables, ln_trivial)
    if key not in _CACHE:
        _CACHE[key] = build_kernel(plan["v2e_gks"], plan["e2v_gks"],
                                   plan["v2e_T"], plan["e2v_T"],
                                   plan["v2e_M"], plan["e2v_M"],
                                   debug_tables=debug_tables,
                                   ln_trivial=ln_trivial)
    nc = _CACHE[key]

    in_maps = prepare_inputs(X, edge_idx, vertex_idx, theta_w, theta_b,
                             atten_w, ln_w, ln_b, conv_w, conv_b, gamma, plan)
    res = run_bass_kernel_spmd(nc, in_maps, core_ids=list(range(NCORES)),
                               trace=trace)
    kernel.last_results = res
    outs = []
    for c in range(NCORES):
        o = np.asarray(res.results[c]["out"]).astype(np.float32)
        outs.append(o[plan["perms"][c]])
    return np.concatenate(outs, axis=0)

